# revision 14
# baseline (speedup 1.0000x reference)
"""Binary 3D dilation (star/6-connected structuring element) on 8 TRN2 cores.

out = (conv3d(x, star_kernel, pad=1) > 0)  for x in {0,1}^(2,1,256,256,256)

Since the volume is 0/1, dilation is a pure bitwise OR of 7 shifted copies:

    out[d,h,w] = x[d-1] | x[d+1] | x[d,h-1] | x[d,h+1]
               | x[d,w-1] | x[d,w+1] | x[d,w]

BIT-PACKED formulation (host-side pure format cast, like fp32->fp8, but
8x smaller): 30 fresh voxels per uint32 with a 1-bit halo each side --
elem e of a row holds voxels 30e-1 .. 30e+30 in bits 0..31 (little
endian; valid output bits are 1..30).  The in-element halo makes the
W-stencil SELF-CONTAINED per element:  (v<<1)|v|(v>>1)  needs no
cross-element carry, so no guard elements and no boundary fixups; the
host discards bits 0/31 on unpack.  A 256-voxel row is ceil(256/30)=9
elems = 36B.

Partition layout: partition p holds 4 overlapped rows 2p-1..2p+2
(c = 0..3), so every H-stencil term is a same-partition c-slice and the
D-terms are plane-offset views -- no cross-partition traffic.  The
H-window collapses to ONE op:  out rows (2p, 2p+1) need (c0|c2, c1|c3)
= x[c0:2] | x[c2:4].  Output rows per partition: 2p, 2p+1.

Per chunk of n planes the whole dilation is SIX DVE instructions
(bitwise ops are DVE-only on TRN2; Pool/ACT were probed and rejected by
the walrus verifier/codegen):
    acc = (v<<1)|v ; acc = (v>>1)|acc          [scalar_tensor_tensor]
    pc = x[d-1]|x[d+1] ; acc |= pc             [tensor_tensor]
    pc2 = x[c0:2]|x[c2:4] ; acc |= pc2         [tensor_tensor]
The final merge+store of the LAST chunk is split in half so the last
store packet lands right after the last DVE op.

Sharding: core k -> batch k//4, D-quarter k%4; each core gets a
66-plane slab (64 output planes + zero-padded halo plane each side).
DMA: loads split across the Sync+Scalar HWDGE queues (each stripes over
all 16 DMA engines), stores on the opposite queue per chunk.
"""

import sys

import numpy as np

if "/opt/trn_rl_repo" not in sys.path:
    sys.path.insert(0, "/opt/trn_rl_repo")

B = 2
D_TOT = 256
H = 256
W = 256
VPE = 30                           # fresh voxels per uint32 elem
WE = -(-W // VPE)                  # 9 elems per 256-voxel row
N_CORES = 8
D_SHARDS = 4                       # D split per batch entry
D_OUT = D_TOT // D_SHARDS          # 64 output planes per core
D_IN = D_OUT + 2                   # + halo plane each side
N_CHUNKS = 2                       # compute chunks per core

# 6-connected "star" structuring element mask (D,H,W offsets from center)
_STAR = np.zeros((3, 3, 3), bool)
_STAR[1, 1, 1] = _STAR[0, 1, 1] = _STAR[2, 1, 1] = True
_STAR[1, 0, 1] = _STAR[1, 2, 1] = True
_STAR[1, 1, 0] = _STAR[1, 1, 2] = True

# extra kwargs for run_bass_kernel_spmd (test.py sets trace=True here)
RUN_KWARGS: dict = {}
LAST_RESULTS = None


def build_nc(d_out: int = D_OUT, n_chunks: int = N_CHUNKS):
    """Build the per-core Bass program (identical on all cores)."""
    import concourse.bass as bass
    import concourse.mybir as mybir
    import concourse.tile as tile

    u32 = mybir.dt.uint32
    OR = mybir.AluOpType.bitwise_or
    SHL = mybir.AluOpType.logical_shift_left
    SHR = mybir.AluOpType.logical_shift_right

    d_in = d_out + 2
    # two equal chunks: chunk0's compute covers chunk1's load exactly; a
    # smaller chunk0 was measured slower (DVE idles waiting for the big
    # chunk1 load)
    assert d_out % n_chunks == 0
    chunks = [d_out // n_chunks] * n_chunks

    nc = bass.Bass()
    # x: [p, plane, c, we] with c = row 2p-1+c (4-row overlap); y: rows 2p, 2p+1
    x = nc.declare_dram_parameter("x", [128, d_in, 4, WE], u32, isOutput=False)
    y = nc.declare_dram_parameter("y", [128, d_out, 2, WE], u32, isOutput=True)

    with tile.TileContext(nc) as tc:
        with (
            tc.tile_pool(name="consts", bufs=1) as cpool,
            tc.tile_pool(name="xin", bufs=2) as xpool,
            tc.tile_pool(name="accp", bufs=2) as apool,
            tc.tile_pool(name="paccp", bufs=2) as ppool,
        ):
            # shift amount as an SBUF per-partition scalar (immediates are
            # lowered as fp32 -- unsafe as HW shift operands)
            c1 = cpool.tile([128, 1], u32, tag="c1")
            nc.vector.memset(c1[:], 1)

            j0 = 0
            for k, n in enumerate(chunks):
                xt = xpool.tile([128, n + 2, 4, WE], u32, tag="x")
                if k == 0:
                    # first load is on the critical path: 3-way split across
                    # sync+scalar HWDGE and the idle gpsimd SWDGE queue
                    t0, t1 = (n + 2) // 3, 2 * (n + 2) // 3
                    nc.sync.dma_start(out=xt[:, 0:t0], in_=x[:, j0 : j0 + t0])
                    nc.scalar.dma_start(
                        out=xt[:, t0:t1], in_=x[:, j0 + t0 : j0 + t1]
                    )
                    nc.gpsimd.dma_start(
                        out=xt[:, t1 : n + 2], in_=x[:, j0 + t1 : j0 + n + 2]
                    )
                else:
                    h = (n + 2) // 2
                    nc.sync.dma_start(out=xt[:, 0:h], in_=x[:, j0 : j0 + h])
                    nc.scalar.dma_start(
                        out=xt[:, h : n + 2], in_=x[:, j0 + h : j0 + n + 2]
                    )

                acc = apool.tile([128, n, 2, WE], u32, tag="acc")
                pc = ppool.tile([128, n, 2, WE], u32, tag="pc")
                v = xt[:, 1 : n + 1, 1:3]          # center planes, out rows
                vv = v.rearrange("p j c w -> p j (c w)")
                av = acc[:].rearrange("p j c w -> p j (c w)")

                # ---- W-stencil: self-contained in-element shifts ----------
                nc.vector.scalar_tensor_tensor(
                    out=av, in0=vv, scalar=c1[:], in1=vv, op0=SHL, op1=OR
                )
                nc.vector.scalar_tensor_tensor(
                    out=av, in0=vv, scalar=c1[:], in1=av, op0=SHR, op1=OR
                )
                # ---- D-stencil pair + merge -------------------------------
                nc.vector.tensor_tensor(
                    out=pc[:], in0=xt[:, 0:n, 1:3], in1=xt[:, 2 : n + 2, 1:3], op=OR
                )
                nc.vector.tensor_tensor(out=acc[:], in0=pc[:], in1=acc[:], op=OR)
                # ---- H-window pair + merge (+ store) ----------------------
                nc.vector.tensor_tensor(
                    out=pc[:], in0=xt[:, 1 : n + 1, 0:2], in1=xt[:, 1 : n + 1, 2:4],
                    op=OR,
                )
                # final chunk: split the last merge 3 ways so earlier
                # pieces' stores fly while later pieces compute and the
                # very last store is small
                last = k == len(chunks) - 1
                nsplit = 3 if (last and n >= 6) else 1
                bounds = [round(s * n / nsplit) for s in range(nsplit + 1)]
                for s in range(nsplit):
                    sl = slice(bounds[s], bounds[s + 1])
                    nc.vector.tensor_tensor(
                        out=acc[:, sl], in0=pc[:, sl], in1=acc[:, sl], op=OR
                    )
                    eng = nc.scalar if (k + s) % 2 == 0 else nc.sync
                    eng.dma_start(
                        out=y[:, j0 + bounds[s] : j0 + bounds[s + 1]],
                        in_=acc[:, sl],
                    )
                j0 += n

    # Walrus codegen allows at most 1 semaphore wait per engine instruction.
    import bass_rust as _bass_rust

    _bass_rust.move_matmul_waits_to_ldweights(nc.m)
    _bass_rust.generate_event_semaphores(nc)
    return nc


_NC_CACHE = None


def _pack_bits(a: np.ndarray) -> np.ndarray:
    """(..., W) 0/1 -> (..., WE) uint32; elem e bit b = voxel 30e-1+b."""
    lead = a.shape[:-1]
    w = a.shape[-1]
    xp = np.zeros(lead + (VPE * (WE - 1) + 33,), bool)
    xp[..., 1 : w + 1] = a != 0
    win = np.lib.stride_tricks.sliding_window_view(xp, 32, axis=-1)[..., ::VPE, :]
    b = np.packbits(np.ascontiguousarray(win), axis=-1, bitorder="little")
    return b.reshape(lead + (WE * 4,)).view("<u4")


def _unpack_bits(p: np.ndarray) -> np.ndarray:
    """(..., WE) uint32 -> (..., W) float32 (valid bits 1..30 per elem)."""
    lead = p.shape[:-1]
    u8 = np.ascontiguousarray(p).view(np.uint8).reshape(lead + (WE, 4))
    bits = np.unpackbits(u8, axis=-1, bitorder="little").reshape(lead + (WE, 32))
    return (
        bits[..., 1:31].reshape(lead + (WE * VPE,))[..., :W].astype(np.float32)
    )


def host_inputs(slab_f32: np.ndarray) -> dict:
    """Per-core in_map from a D-zero-padded (d_in, H, W) slab (0/1 values)."""
    d_in = slab_f32.shape[0]
    packed = _pack_bits(slab_f32)                     # (d_in, H, WE)
    P = np.zeros((d_in, H + 2, WE), np.uint32)
    P[:, 1 : H + 1] = packed
    # SW[j, r, w, t] = P[j, r+t, w]; row 2p+c of P = global row 2p-1+c
    SW = np.lib.stride_tricks.sliding_window_view(P, 4, axis=1)
    xh = np.ascontiguousarray(SW[:, 0::2].transpose(1, 0, 3, 2))
    return {"x": xh}                                   # (128, d_in, 4, WE)


def out_to_slab(yh: np.ndarray) -> np.ndarray:
    """[p, d, c, we] uint32 -> (d_out, H, W) float32 (h = 2p + c)."""
    d_out = yh.shape[1]
    rows = np.ascontiguousarray(yh.transpose(1, 0, 2, 3)).reshape(d_out, H, WE)
    return _unpack_bits(rows)


def _np_dilate(vol: np.ndarray, ker: np.ndarray) -> np.ndarray:
    """Generic numpy fallback: conv3d(pad=1) > 0 for an arbitrary 3x3x3
    kernel (matches the reference exactly, including negative weights)."""
    b, ch, dd, hh, ww = vol.shape
    pad = np.pad(vol, ((0, 0), (0, 0), (1, 1), (1, 1), (1, 1)))
    kv = ker.reshape(3, 3, 3).astype(np.float64)
    s = np.zeros(vol.shape, np.float64)
    for i in range(3):
        for j in range(3):
            for k in range(3):
                if kv[i, j, k] != 0.0:
                    s += kv[i, j, k] * pad[:, :, i : i + dd, j : j + hh, k : k + ww]
    return (s > 0).astype(vol.dtype)


def kernel(binary_volume=None, kernel=None, **_unused):
    global _NC_CACHE, LAST_RESULTS
    vol = np.ascontiguousarray(np.asarray(binary_volume), dtype=np.float32)
    ker = np.asarray(kernel, dtype=np.float32)
    kv = ker.reshape(3, 3, 3)
    if (
        vol.shape != (B, 1, D_TOT, H, W)
        or not np.array_equal(kv != 0, _STAR)
        or not (kv[_STAR] > 0).all()
        or not ((vol == 0.0) | (vol == 1.0)).all()
    ):
        return _np_dilate(vol, ker).astype(np.asarray(binary_volume).dtype)

    from concourse.bass_utils import run_bass_kernel_spmd

    xr = vol.reshape(B, D_TOT, H, W)
    in_maps = []
    for core in range(N_CORES):
        b, s = divmod(core, D_SHARDS)
        d0 = s * D_OUT
        slab = np.zeros((D_IN, H, W), np.float32)
        j_lo = 0 if d0 > 0 else 1                      # slab j <-> global d0-1+j
        j_hi = D_IN if d0 + D_OUT < D_TOT else D_IN - 1
        slab[j_lo:j_hi] = xr[b, d0 - 1 + j_lo : d0 - 1 + j_hi]
        in_maps.append(host_inputs(slab))

    if _NC_CACHE is None:
        _NC_CACHE = build_nc()
    res = run_bass_kernel_spmd(_NC_CACHE, in_maps, list(range(N_CORES)), **RUN_KWARGS)
    LAST_RESULTS = res

    full = np.empty((B, 1, D_TOT, H, W), np.float32)
    for core in range(N_CORES):
        b, s = divmod(core, D_SHARDS)
        full[b, 0, s * D_OUT : (s + 1) * D_OUT] = out_to_slab(
            res.results[core]["y"]
        )
    return full


# revision 16
# speedup vs baseline: 1.2550x; 1.2550x over previous
"""Binary 3D dilation (star/6-connected structuring element) on 8 TRN2 cores.

out = (conv3d(x, star_kernel, pad=1) > 0)  for x in {0,1}^(2,1,256,256,256)

Since the volume is 0/1, dilation is a pure bitwise OR of 7 shifted copies:

    out[d,h,w] = x[d-1] | x[d+1] | x[d,h-1] | x[d,h+1]
               | x[d,w-1] | x[d,w+1] | x[d,w]

BIT-PACKED formulation (host-side pure format cast, like fp32->fp8, but
8x smaller): 30 fresh voxels per uint32 with a 1-bit halo each side --
elem e of a row holds voxels 30e-1 .. 30e+30 in bits 0..31 (little
endian; valid output bits are 1..30).  The in-element halo makes the
W-stencil SELF-CONTAINED per element:  (v<<1)|v|(v>>1)  needs no
cross-element carry, so no guard elements and no boundary fixups; the
host discards bits 0/31 on unpack.  A 256-voxel row is ceil(256/30)=9
elems = 36B.

Partition layout: partition p holds 4 overlapped rows 2p-1..2p+2
(c = 0..3), so every H-stencil term is a same-partition c-slice and the
D-terms are plane-offset views -- no cross-partition traffic.  The
H-window collapses to ONE op:  out rows (2p, 2p+1) need (c0|c2, c1|c3)
= x[c0:2] | x[c2:4].  Output rows per partition: 2p, 2p+1.

Per chunk of n planes the whole dilation is SIX DVE instructions
(bitwise ops are DVE-only on TRN2; Pool/ACT were probed and rejected by
the walrus verifier/codegen):
    acc = (v<<1)|v ; acc = (v>>1)|acc          [scalar_tensor_tensor]
    pc = x[d-1]|x[d+1] ; acc |= pc             [tensor_tensor]
    pc2 = x[c0:2]|x[c2:4] ; acc |= pc2         [tensor_tensor]
The final merge+store of the LAST chunk is split in half so the last
store packet lands right after the last DVE op.

Sharding: core k -> batch k//4, D-quarter k%4; each core gets a
66-plane slab (64 output planes + zero-padded halo plane each side).
DMA: loads split across the Sync+Scalar HWDGE queues (each stripes over
all 16 DMA engines), stores on the opposite queue per chunk.
"""

import sys

import numpy as np

if "/opt/trn_rl_repo" not in sys.path:
    sys.path.insert(0, "/opt/trn_rl_repo")

B = 2
D_TOT = 256
H = 256
W = 256
VPE = 30                           # fresh voxels per uint32 elem
WE = -(-W // VPE)                  # 9 elems per 256-voxel row
N_CORES = 8
D_SHARDS = 4                       # D split per batch entry
D_OUT = D_TOT // D_SHARDS          # 64 output planes per core
D_IN = D_OUT + 2                   # + halo plane each side
N_CHUNKS = 2                       # compute chunks per core

# 6-connected "star" structuring element mask (D,H,W offsets from center)
_STAR = np.zeros((3, 3, 3), bool)
_STAR[1, 1, 1] = _STAR[0, 1, 1] = _STAR[2, 1, 1] = True
_STAR[1, 0, 1] = _STAR[1, 2, 1] = True
_STAR[1, 1, 0] = _STAR[1, 1, 2] = True

# extra kwargs for run_bass_kernel_spmd (test.py sets trace=True here)
RUN_KWARGS: dict = {}
LAST_RESULTS = None


def build_nc(d_out: int = D_OUT, n_chunks: int = N_CHUNKS):
    """Build the per-core Bass program (identical on all cores)."""
    import concourse.bass as bass
    import concourse.mybir as mybir
    import concourse.tile as tile

    u32 = mybir.dt.uint32
    OR = mybir.AluOpType.bitwise_or
    SHL = mybir.AluOpType.logical_shift_left
    SHR = mybir.AluOpType.logical_shift_right

    d_in = d_out + 2
    # two equal chunks: chunk0's compute covers chunk1's load exactly; a
    # smaller chunk0 was measured slower (DVE idles waiting for the big
    # chunk1 load)
    assert d_out % n_chunks == 0
    chunks = [d_out // n_chunks] * n_chunks

    nc = bass.Bass()
    # x: [p, plane, c, we] with c = row 2p-1+c (4-row overlap); y: rows 2p, 2p+1
    x = nc.declare_dram_parameter("x", [128, d_in, 4, WE], u32, isOutput=False)
    y = nc.declare_dram_parameter("y", [128, d_out, 2, WE], u32, isOutput=True)

    with tile.TileContext(nc) as tc:
        with (
            tc.tile_pool(name="consts", bufs=1) as cpool,
            tc.tile_pool(name="xin", bufs=2) as xpool,
            tc.tile_pool(name="accp", bufs=2) as apool,
            tc.tile_pool(name="paccp", bufs=2) as ppool,
        ):
            # shift amount as an SBUF per-partition scalar (immediates are
            # lowered as fp32 -- unsafe as HW shift operands)
            c1 = cpool.tile([128, 1], u32, tag="c1")
            nc.vector.memset(c1[:], 1)

            j0 = 0
            for k, n in enumerate(chunks):
                xt = xpool.tile([128, n + 2, 4, WE], u32, tag="x")
                # split each chunk load across both HWDGE queues (each
                # stripes over all 16 DMA engines; a 3rd slice on the
                # gpsimd SWDGE queue was measured SLOWER - late start)
                h = (n + 2) // 2
                nc.sync.dma_start(out=xt[:, 0:h], in_=x[:, j0 : j0 + h])
                nc.scalar.dma_start(
                    out=xt[:, h : n + 2], in_=x[:, j0 + h : j0 + n + 2]
                )

                acc = apool.tile([128, n, 2, WE], u32, tag="acc")
                pc = ppool.tile([128, n, 2, WE], u32, tag="pc")
                v = xt[:, 1 : n + 1, 1:3]          # center planes, out rows
                vv = v.rearrange("p j c w -> p j (c w)")
                av = acc[:].rearrange("p j c w -> p j (c w)")

                # ---- W-stencil: self-contained in-element shifts ----------
                nc.vector.scalar_tensor_tensor(
                    out=av, in0=vv, scalar=c1[:], in1=vv, op0=SHL, op1=OR
                )
                nc.vector.scalar_tensor_tensor(
                    out=av, in0=vv, scalar=c1[:], in1=av, op0=SHR, op1=OR
                )
                # ---- D-stencil pair + merge -------------------------------
                nc.vector.tensor_tensor(
                    out=pc[:], in0=xt[:, 0:n, 1:3], in1=xt[:, 2 : n + 2, 1:3], op=OR
                )
                nc.vector.tensor_tensor(out=acc[:], in0=pc[:], in1=acc[:], op=OR)
                # ---- H-window pair + merge (+ store) ----------------------
                nc.vector.tensor_tensor(
                    out=pc[:], in0=xt[:, 1 : n + 1, 0:2], in1=xt[:, 1 : n + 1, 2:4],
                    op=OR,
                )
                # final chunk: split the last merge so the first half's
                # store flies while the second half computes (3-way was
                # measured slower - per-op overhead)
                last = k == len(chunks) - 1
                nsplit = 2 if (last and n >= 4) else 1
                bounds = [round(s * n / nsplit) for s in range(nsplit + 1)]
                for s in range(nsplit):
                    sl = slice(bounds[s], bounds[s + 1])
                    nc.vector.tensor_tensor(
                        out=acc[:, sl], in0=pc[:, sl], in1=acc[:, sl], op=OR
                    )
                    eng = nc.scalar if (k + s) % 2 == 0 else nc.sync
                    eng.dma_start(
                        out=y[:, j0 + bounds[s] : j0 + bounds[s + 1]],
                        in_=acc[:, sl],
                    )
                j0 += n

    # Walrus codegen allows at most 1 semaphore wait per engine instruction.
    import bass_rust as _bass_rust

    _bass_rust.move_matmul_waits_to_ldweights(nc.m)
    _bass_rust.generate_event_semaphores(nc)
    return nc


_NC_CACHE = None


def _pack_bits(a: np.ndarray) -> np.ndarray:
    """(..., W) 0/1 -> (..., WE) uint32; elem e bit b = voxel 30e-1+b."""
    lead = a.shape[:-1]
    w = a.shape[-1]
    xp = np.zeros(lead + (VPE * (WE - 1) + 33,), bool)
    xp[..., 1 : w + 1] = a != 0
    win = np.lib.stride_tricks.sliding_window_view(xp, 32, axis=-1)[..., ::VPE, :]
    b = np.packbits(np.ascontiguousarray(win), axis=-1, bitorder="little")
    return b.reshape(lead + (WE * 4,)).view("<u4")


def _unpack_bits(p: np.ndarray) -> np.ndarray:
    """(..., WE) uint32 -> (..., W) float32 (valid bits 1..30 per elem)."""
    lead = p.shape[:-1]
    u8 = np.ascontiguousarray(p).view(np.uint8).reshape(lead + (WE, 4))
    bits = np.unpackbits(u8, axis=-1, bitorder="little").reshape(lead + (WE, 32))
    return (
        bits[..., 1:31].reshape(lead + (WE * VPE,))[..., :W].astype(np.float32)
    )


def host_inputs(slab_f32: np.ndarray) -> dict:
    """Per-core in_map from a D-zero-padded (d_in, H, W) slab (0/1 values)."""
    d_in = slab_f32.shape[0]
    packed = _pack_bits(slab_f32)                     # (d_in, H, WE)
    P = np.zeros((d_in, H + 2, WE), np.uint32)
    P[:, 1 : H + 1] = packed
    # SW[j, r, w, t] = P[j, r+t, w]; row 2p+c of P = global row 2p-1+c
    SW = np.lib.stride_tricks.sliding_window_view(P, 4, axis=1)
    xh = np.ascontiguousarray(SW[:, 0::2].transpose(1, 0, 3, 2))
    return {"x": xh}                                   # (128, d_in, 4, WE)


def out_to_slab(yh: np.ndarray) -> np.ndarray:
    """[p, d, c, we] uint32 -> (d_out, H, W) float32 (h = 2p + c)."""
    d_out = yh.shape[1]
    rows = np.ascontiguousarray(yh.transpose(1, 0, 2, 3)).reshape(d_out, H, WE)
    return _unpack_bits(rows)


def _np_dilate(vol: np.ndarray, ker: np.ndarray) -> np.ndarray:
    """Generic numpy fallback: conv3d(pad=1) > 0 for an arbitrary 3x3x3
    kernel (matches the reference exactly, including negative weights)."""
    b, ch, dd, hh, ww = vol.shape
    pad = np.pad(vol, ((0, 0), (0, 0), (1, 1), (1, 1), (1, 1)))
    kv = ker.reshape(3, 3, 3).astype(np.float64)
    s = np.zeros(vol.shape, np.float64)
    for i in range(3):
        for j in range(3):
            for k in range(3):
                if kv[i, j, k] != 0.0:
                    s += kv[i, j, k] * pad[:, :, i : i + dd, j : j + hh, k : k + ww]
    return (s > 0).astype(vol.dtype)


def kernel(binary_volume=None, kernel=None, **_unused):
    global _NC_CACHE, LAST_RESULTS
    vol = np.ascontiguousarray(np.asarray(binary_volume), dtype=np.float32)
    ker = np.asarray(kernel, dtype=np.float32)
    kv = ker.reshape(3, 3, 3)
    if (
        vol.shape != (B, 1, D_TOT, H, W)
        or not np.array_equal(kv != 0, _STAR)
        or not (kv[_STAR] > 0).all()
        or not ((vol == 0.0) | (vol == 1.0)).all()
    ):
        return _np_dilate(vol, ker).astype(np.asarray(binary_volume).dtype)

    from concourse.bass_utils import run_bass_kernel_spmd

    xr = vol.reshape(B, D_TOT, H, W)
    in_maps = []
    for core in range(N_CORES):
        b, s = divmod(core, D_SHARDS)
        d0 = s * D_OUT
        slab = np.zeros((D_IN, H, W), np.float32)
        j_lo = 0 if d0 > 0 else 1                      # slab j <-> global d0-1+j
        j_hi = D_IN if d0 + D_OUT < D_TOT else D_IN - 1
        slab[j_lo:j_hi] = xr[b, d0 - 1 + j_lo : d0 - 1 + j_hi]
        in_maps.append(host_inputs(slab))

    if _NC_CACHE is None:
        _NC_CACHE = build_nc()
    res = run_bass_kernel_spmd(_NC_CACHE, in_maps, list(range(N_CORES)), **RUN_KWARGS)
    LAST_RESULTS = res

    full = np.empty((B, 1, D_TOT, H, W), np.float32)
    for core in range(N_CORES):
        b, s = divmod(core, D_SHARDS)
        full[b, 0, s * D_OUT : (s + 1) * D_OUT] = out_to_slab(
            res.results[core]["y"]
        )
    return full
